# revision 9
# baseline (speedup 1.0000x reference)
"""GQA attention block (RoPE + causal softmax + out-projection) on 8 TRN2 cores.

Problem: q (2, 2048, 1024) 16 heads, k/v (2, 2048, 256) 4 kv heads (GQA rep 4),
causal attention, out @ w_out (1024, 1024).

Sharding: core c = (batch b = c//4, kv group = c%4). Each core computes its 4
q-heads x full T attention against its kv head, then the partial projection
X_heads @ w_out[head_rows, :]; the host sums the 4 partials per batch.

Layout trick: everything is computed transposed (channels on partitions,
sequence on the free axis):
  - S^T = K^T-block (64,128) stationary  @  Q^T (64, 512q) moving   (k on psum partitions)
  - P^T = exp(S^T/8) with causal handled by skipping kb blocks + one masked
    diagonal 128x128 add; no row-max (logits are O(1)) and no P normalization
  - O^T accumulates [V | 1] (128k, 65) stationary @ P^T moving, so the softmax
    denominator falls out as psum row 64; normalization = reciprocal +
    ones-matmul partition-broadcast + one multiply per (head, 512q)
  - projection: w-chunk (128c, 128n) stationary @ X^T (128c, 512q) moving,
    psum DMA'd straight to DRAM as out^T; host transposes while gathering.
RoPE runs on-chip: rotate_half is a signed-permutation matmul, the sin/cos
combine is 3 vector ops per 512-chunk against host-built per-partition tables.
All matmuls use float32r (1 cycle/row at N>=256, ~tf32 precision).
"""

import sys

if "/opt/trn_rl_repo" not in sys.path:
    sys.path.insert(0, "/opt/trn_rl_repo")

import numpy as np

B, T, D, NH, NKV, HD = 2, 2048, 1024, 16, 4, 64
HC = NH // NKV          # q heads per core = 4
CD = HC * HD            # per-core channel dim = 256
KVD = HD                # per-core kv channel dim = 64
NCORES = 8
QB = 128                # q/k block
NG = T // 512           # q column groups of 512
NKB = T // QB           # k blocks = 16
MASK = -240.0           # pre-scale additive mask; exp(-240/8) = exp(-30) ~ 1e-13

_cache: dict = {}


def _tables():
    if "tables" in _cache:
        return _cache["tables"]
    p = np.arange(128)
    t = np.arange(T)
    ang = t[None, :] / (10000.0 ** ((p[:, None] % 32) / 32.0))
    cosT = np.cos(ang).astype(np.float32)
    sinT = np.sin(ang).astype(np.float32)

    rotP = np.zeros((128, 128), np.float32)
    for base in (0, 64):
        for i in range(32):
            rotP[base + 32 + i, base + i] = -1.0   # out[i] = -x[i+32]
            rotP[base + i, base + 32 + i] = 1.0    # out[i+32] = x[i]

    kk = np.arange(QB)
    maskT = np.where(kk[:, None] <= kk[None, :], 0.0, MASK).astype(np.float32)
    _cache["tables"] = (cosT, sinT, rotP, maskT)
    return _cache["tables"]


def _build():
    import concourse.tile as tile
    from concourse import bacc, mybir

    f32 = mybir.dt.float32
    f32r = mybir.dt.float32r
    Exp = mybir.ActivationFunctionType.Exp

    nc = bacc.Bacc("TRN2", target_bir_lowering=False, debug=False,
                   num_devices=NCORES)

    d_qT = nc.dram_tensor("qT", [CD, T], f32r, kind="ExternalInput")
    d_kT = nc.dram_tensor("kT", [KVD, T], f32r, kind="ExternalInput")
    d_vaug = nc.dram_tensor("vaug", [T, HD + 1], f32r, kind="ExternalInput")
    d_w = nc.dram_tensor("w", [CD, D], f32r, kind="ExternalInput")
    d_cosT = nc.dram_tensor("cosT", [128, T], f32, kind="ExternalInput")
    d_sinT = nc.dram_tensor("sinT", [128, T], f32, kind="ExternalInput")
    d_rotP = nc.dram_tensor("rotP", [128, 128], f32r, kind="ExternalInput")
    d_maskT = nc.dram_tensor("maskT", [QB, QB], f32, kind="ExternalInput")
    d_ones = nc.dram_tensor("ones64", [1, 64], f32r, kind="ExternalInput")
    d_outT = nc.dram_tensor("outT", [D, T], f32, kind="ExternalOutput")

    with tile.TileContext(nc) as tc:
        with (
            tc.tile_pool(name="consts", bufs=1) as consts,
            tc.tile_pool(name="data", bufs=1) as data,
            tc.tile_pool(name="pt", bufs=3) as ptp,
            tc.tile_pool(name="small", bufs=3) as small,
            tc.tile_pool(name="psS", bufs=2, space="PSUM") as psS,
            tc.tile_pool(name="psO", bufs=2, space="PSUM") as psO,
            tc.tile_pool(name="psB", bufs=1, space="PSUM") as psB,
            tc.tile_pool(name="ps512", bufs=3, space="PSUM") as ps512,
        ):
            cosT = consts.tile([128, T], f32)
            sinT = consts.tile([128, T], f32)
            rotP = consts.tile([128, 128], f32r)
            maskT = consts.tile([QB, QB], f32)
            ones64 = consts.tile([1, 64], f32r)
            nc.sync.dma_start(cosT[:], d_cosT[:])
            nc.sync.dma_start(sinT[:], d_sinT[:])
            nc.sync.dma_start(rotP[:], d_rotP[:])
            nc.sync.dma_start(maskT[:], d_maskT[:])
            nc.sync.dma_start(ones64[:], d_ones[:])

            qT = [data.tile([128, T], f32r, name=f"qT{i}", tag=f"qT{i}")
                  for i in range(2)]
            for i in range(2):
                nc.sync.dma_start(qT[i][:], d_qT[i * 128:(i + 1) * 128, :])
            kT = data.tile([KVD, T], f32r, tag="kT")
            nc.sync.dma_start(kT[:], d_kT[:])
            vaug = data.tile([128, NKB, HD + 1], f32r, tag="vaug")
            nc.sync.dma_start(
                vaug[:], d_vaug[:].rearrange("(n p) m -> p n m", p=128))
            w = [data.tile([128, D], f32r, name=f"w{i}", tag=f"w{i}")
                 for i in range(2)]
            for i in range(2):
                nc.sync.dma_start(w[i][:], d_w[i * 128:(i + 1) * 128, :])
            xT = [data.tile([128, T], f32r, name=f"xT{i}", tag=f"xT{i}")
                  for i in range(2)]

            # ---- RoPE (in-place on qT tiles and kT) ----
            def rope(dst, rows, tab_rows):
                for f in range(T // 512):
                    sl = slice(f * 512, (f + 1) * 512)
                    rot = ps512.tile([rows, 512], f32, tag="mm512")
                    nc.tensor.matmul(
                        rot[:], rotP[:rows, :rows],
                        dst[:, sl], start=True, stop=True)
                    nc.vector.tensor_mul(rot[:], rot[:], sinT[:tab_rows, sl])
                    nc.vector.tensor_mul(dst[:, sl], dst[:, sl],
                                         cosT[:tab_rows, sl])
                    nc.vector.tensor_add(dst[:, sl], dst[:, sl], rot[:])

            rope(qT[0], 128, 128)
            rope(qT[1], 128, 128)
            rope(kT, KVD, KVD)

            # second copy of roped K^T at partition base 64: matmul operands
            # must share a base partition, and odd heads' Q^T rows sit at 64
            kThi = data.tile([128, T], f32r, tag="kThi")
            nc.vector.tensor_copy(kThi[64:128, :], kT[:])

            # ---- attention per (head, 512-wide q group) ----
            for h in range(HC):
                qoff = (h % 2) * 64
                kTh = kT if qoff == 0 else kThi[64:128, :]
                for g in range(NG):
                    oT = psO.tile([HD + 1, 512], f32)
                    nkb = 4 * g + 4
                    for kb in range(nkb):
                        cs = QB * max(0, kb - 4 * g)
                        S = psS.tile([128, 512], f32)
                        nc.tensor.matmul(
                            S[:, cs:],
                            kTh[:, kb * QB:(kb + 1) * QB],
                            qT[h // 2][qoff:qoff + 64,
                                       g * 512 + cs:(g + 1) * 512],
                            start=True, stop=True)
                        if kb >= 4 * g:
                            nc.vector.tensor_add(S[:, cs:cs + QB],
                                                 S[:, cs:cs + QB], maskT[:])
                        PT = ptp.tile([128, 512], f32r)
                        nc.scalar.activation(PT[:, cs:], S[:, cs:], Exp,
                                             scale=0.125)
                        nc.tensor.matmul(
                            oT[:, cs:], vaug[:, kb, :],
                            PT[:, cs:],
                            start=(kb == 0), stop=(kb == nkb - 1),
                            skip_group_check=True)
                    # normalize by psum row 64 (softmax denominator)
                    den = small.tile([1, 512], f32r, tag="den")
                    with nc.allow_low_precision("f32r softmax denom"):
                        nc.vector.reciprocal(den[:], oT[HD:HD + 1, :])
                    bc = psB.tile([64, 512], f32)
                    nc.tensor.matmul(bc[:], ones64[:],
                                     den[:], start=True, stop=True)
                    bcs = small.tile([64, 512], f32, tag="bcs")
                    nc.vector.tensor_copy(bcs[:], bc[:])
                    nc.vector.tensor_mul(
                        xT[h // 2][qoff:qoff + 64, g * 512:(g + 1) * 512],
                        oT[:HD, :], bcs[:])

            # ---- projection: out^T[n, q] += w[c, n]^T-chunks @ x^T[c, q] ----
            for n in range(D // 128):
                for g in range(NG):
                    pr = ps512.tile([128, 512], f32, tag="mm512")
                    for cc in range(2):
                        nc.tensor.matmul(
                            pr[:],
                            w[cc][:, n * 128:(n + 1) * 128],
                            xT[cc][:, g * 512:(g + 1) * 512],
                            start=(cc == 0), stop=(cc == 1))
                    st = ptp.tile([128, 512], f32, tag="st")
                    nc.scalar.copy(st[:], pr[:])
                    nc.sync.dma_start(
                        d_outT[n * 128:(n + 1) * 128, g * 512:(g + 1) * 512],
                        st[:])

    nc.finalize()
    return nc


def _get_nc():
    if "nc" not in _cache:
        _cache["nc"] = _build()
    return _cache["nc"]


def _in_maps(q, k, v, w_out):
    cosT, sinT, rotP, maskT = _tables()
    ones = np.ones((T, 1), np.float32)
    maps = []
    for c in range(NCORES):
        b, kv = divmod(c, NKV)
        maps.append({
            "qT": np.ascontiguousarray(q[b, :, kv * CD:(kv + 1) * CD].T),
            "kT": np.ascontiguousarray(k[b, :, kv * KVD:(kv + 1) * KVD].T),
            "vaug": np.ascontiguousarray(
                np.concatenate([v[b, :, kv * KVD:(kv + 1) * KVD], ones], 1)),
            "w": np.ascontiguousarray(w_out[kv * CD:(kv + 1) * CD, :]),
            "cosT": cosT, "sinT": sinT, "rotP": rotP, "maskT": maskT,
            "ones64": np.ones((1, 64), np.float32),
        })
    return maps


def _run(q, k, v, w_out, trace=False):
    from concourse.bass_utils import run_bass_kernel_spmd

    nc = _get_nc()
    res = run_bass_kernel_spmd(nc, _in_maps(q, k, v, w_out),
                               core_ids=list(range(NCORES)), trace=trace)
    out = np.zeros((B, T, D), np.float32)
    for c in range(NCORES):
        out[c // NKV] += res.results[c]["outT"].T
    return out, res


def kernel(q, k, v, w_out):
    out, _ = _run(np.asarray(q), np.asarray(k), np.asarray(v),
                  np.asarray(w_out))
    return out


# revision 12
# speedup vs baseline: 1.0674x; 1.0674x over previous
"""GQA attention block (RoPE + causal softmax + out-projection) on 8 TRN2 cores.

Problem: q (2, 2048, 1024) 16 heads, k/v (2, 2048, 256) 4 kv heads (GQA rep 4),
causal attention, out @ w_out (1024, 1024).

Sharding: core c = (batch b = c//4, kv group = c%4). Each core computes its 4
q-heads x full T attention against its kv head, then the partial projection
X_heads @ w_out[head_rows, :]; the host sums the 4 partials per batch.

Layout trick: everything is computed transposed (channels on partitions,
sequence on the free axis):
  - S^T = K^T-block (64,128) stationary  @  Q^T (64, 512q) moving   (k on psum partitions)
  - P^T = exp(S^T/8) with causal handled by skipping kb blocks + one masked
    diagonal 128x128 add; no row-max (logits are O(1)) and no P normalization
  - O^T accumulates [V | 1] (128k, 65) stationary @ P^T moving, so the softmax
    denominator falls out as psum row 64; normalization = reciprocal +
    ones-matmul partition-broadcast + one multiply per (head, 512q)
  - projection: w-chunk (128c, 128n) stationary @ X^T (128c, 512q) moving,
    psum DMA'd straight to DRAM as out^T; host transposes while gathering.
RoPE runs on-chip: rotate_half is a signed-permutation matmul, the sin/cos
combine is 3 vector ops per 512-chunk against host-built per-partition tables.
All matmuls use float32r (1 cycle/row at N>=256, ~tf32 precision).
"""

import sys

if "/opt/trn_rl_repo" not in sys.path:
    sys.path.insert(0, "/opt/trn_rl_repo")

import numpy as np

B, T, D, NH, NKV, HD = 2, 2048, 1024, 16, 4, 64
HC = NH // NKV          # q heads per core = 4
CD = HC * HD            # per-core channel dim = 256
KVD = HD                # per-core kv channel dim = 64
NCORES = 8
QB = 128                # q/k block
NG = T // 512           # q column groups of 512
NKB = T // QB           # k blocks = 16
MASK = -240.0           # pre-scale additive mask; exp(-240/8) = exp(-30) ~ 1e-13

_cache: dict = {}


def _tables():
    if "tables" in _cache:
        return _cache["tables"]
    p = np.arange(128)
    t = np.arange(T)
    ang = t[None, :] / (10000.0 ** ((p[:, None] % 32) / 32.0))
    cosT = np.cos(ang).astype(np.float32)
    sinT = np.sin(ang).astype(np.float32)

    rotP = np.zeros((128, 128), np.float32)
    for base in (0, 64):
        for i in range(32):
            rotP[base + 32 + i, base + i] = -1.0   # out[i] = -x[i+32]
            rotP[base + i, base + 32 + i] = 1.0    # out[i+32] = x[i]

    kk = np.arange(QB)
    maskT = np.where(kk[:, None] <= kk[None, :], 0.0, MASK).astype(np.float32)
    _cache["tables"] = (cosT, sinT, rotP, maskT)
    return _cache["tables"]


def _build():
    import concourse.tile as tile
    from concourse import bacc, mybir

    f32 = mybir.dt.float32
    f32r = mybir.dt.float32r
    Exp = mybir.ActivationFunctionType.Exp

    nc = bacc.Bacc("TRN2", target_bir_lowering=False, debug=False,
                   num_devices=NCORES)

    d_qT = nc.dram_tensor("qT", [CD, T], f32r, kind="ExternalInput")
    d_kT = nc.dram_tensor("kT", [KVD, T], f32r, kind="ExternalInput")
    d_vaug = nc.dram_tensor("vaug", [T, HD + 1], f32r, kind="ExternalInput")
    d_w = nc.dram_tensor("w", [CD, D], f32r, kind="ExternalInput")
    d_cosT = nc.dram_tensor("cosT", [128, T], f32, kind="ExternalInput")
    d_sinT = nc.dram_tensor("sinT", [128, T], f32, kind="ExternalInput")
    d_rotP = nc.dram_tensor("rotP", [128, 128], f32r, kind="ExternalInput")
    d_maskT = nc.dram_tensor("maskT", [QB, QB], f32, kind="ExternalInput")
    d_ones = nc.dram_tensor("ones64", [1, 64], f32r, kind="ExternalInput")
    d_outT = nc.dram_tensor("outT", [D, T], f32, kind="ExternalOutput")

    with tile.TileContext(nc) as tc:
        with (
            tc.tile_pool(name="consts", bufs=1) as consts,
            tc.tile_pool(name="data", bufs=1) as data,
            tc.tile_pool(name="pt", bufs=4) as ptp,
            tc.tile_pool(name="small", bufs=3) as small,
            tc.tile_pool(name="psS", bufs=3, space="PSUM") as psS,
            tc.tile_pool(name="psO", bufs=2, space="PSUM") as psO,
            tc.tile_pool(name="psB", bufs=1, space="PSUM") as psB,
        ):
            ps512 = psS  # rope/projection psum shares the S-bank slots
            cosT = consts.tile([128, T], f32)
            sinT = consts.tile([128, T], f32)
            rotP = consts.tile([128, 128], f32r)
            maskT = consts.tile([QB, QB], f32)
            ones64 = consts.tile([1, 64], f32r)
            nc.sync.dma_start(cosT[:], d_cosT[:])
            nc.sync.dma_start(sinT[:], d_sinT[:])
            nc.sync.dma_start(rotP[:], d_rotP[:])
            nc.sync.dma_start(maskT[:], d_maskT[:])
            nc.sync.dma_start(ones64[:], d_ones[:])

            qT = [data.tile([128, T], f32r, name=f"qT{i}", tag=f"qT{i}")
                  for i in range(2)]
            for i in range(2):
                nc.sync.dma_start(qT[i][:], d_qT[i * 128:(i + 1) * 128, :])
            kT = data.tile([KVD, T], f32r, tag="kT")
            nc.sync.dma_start(kT[:], d_kT[:])
            vaug = data.tile([128, NKB, HD + 1], f32r, tag="vaug")
            nc.sync.dma_start(
                vaug[:], d_vaug[:].rearrange("(n p) m -> p n m", p=128))
            w = [data.tile([128, D], f32r, name=f"w{i}", tag=f"w{i}")
                 for i in range(2)]
            for i in range(2):
                nc.sync.dma_start(w[i][:], d_w[i * 128:(i + 1) * 128, :])
            xT = [data.tile([128, T], f32r, name=f"xT{i}", tag=f"xT{i}")
                  for i in range(2)]

            # ---- RoPE (in-place on qT tiles and kT) ----
            def rope(dst, rows, tab_rows):
                for f in range(T // 512):
                    sl = slice(f * 512, (f + 1) * 512)
                    rot = ps512.tile([rows, 512], f32, tag="S")
                    nc.tensor.matmul(
                        rot[:], rotP[:rows, :rows],
                        dst[:, sl], start=True, stop=True)
                    nc.vector.tensor_mul(rot[:], rot[:], sinT[:tab_rows, sl])
                    nc.vector.tensor_mul(dst[:, sl], dst[:, sl],
                                         cosT[:tab_rows, sl])
                    nc.vector.tensor_add(dst[:, sl], dst[:, sl], rot[:])

            rope(qT[0], 128, 128)
            rope(qT[1], 128, 128)
            rope(kT, KVD, KVD)

            # second copy of roped K^T at partition base 64: matmul operands
            # must share a base partition, and odd heads' Q^T rows sit at 64
            kThi = data.tile([128, T], f32r, tag="kThi")
            nc.vector.tensor_copy(kThi[64:128, :], kT[:])

            # ---- attention: head pairs x 512-wide q groups, k-block inner
            # (two independent heads per k-block keep PE/ACT/DVE pipelined)
            for hp in range(HC // 2):
                heads = (2 * hp, 2 * hp + 1)
                for g in range(NG):
                    oTs = {h: psO.tile([HD + 1, 512], f32, name=f"oT{h}",
                                        tag=f"oT{h % 2}")
                           for h in heads}
                    nkb = 4 * g + 4
                    for kb in range(nkb):
                        cs = QB * max(0, kb - 4 * g)
                        for h in heads:
                            qoff = (h % 2) * 64
                            kTh = kT if qoff == 0 else kThi[64:128, :]
                            oT = oTs[h]
                            S = psS.tile([128, 512], f32)
                            nc.tensor.matmul(
                                S[:, cs:],
                                kTh[:, kb * QB:(kb + 1) * QB],
                                qT[h // 2][qoff:qoff + 64,
                                           g * 512 + cs:(g + 1) * 512],
                                start=True, stop=True)
                            if kb >= 4 * g:
                                nc.vector.tensor_add(S[:, cs:cs + QB],
                                                     S[:, cs:cs + QB],
                                                     maskT[:])
                            PT = ptp.tile([128, 512], f32r)
                            nc.scalar.activation(PT[:, cs:], S[:, cs:], Exp,
                                                 scale=0.125)
                            nc.tensor.matmul(
                                oT[:, cs:], vaug[:, kb, :],
                                PT[:, cs:],
                                start=(kb == 0), stop=(kb == nkb - 1),
                                skip_group_check=True)
                    for h in heads:
                        qoff = (h % 2) * 64
                        oT = oTs[h]
                        # normalize by psum row 64 (softmax denominator)
                        den = small.tile([1, 512], f32r, tag="den")
                        with nc.allow_low_precision("f32r softmax denom"):
                            nc.vector.reciprocal(den[:], oT[HD:HD + 1, :])
                        bc = psB.tile([64, 512], f32)
                        nc.tensor.matmul(bc[:], ones64[:],
                                         den[:], start=True, stop=True)
                        bcs = small.tile([64, 512], f32, tag="bcs")
                        nc.vector.tensor_copy(bcs[:], bc[:])
                        nc.vector.tensor_mul(
                            xT[h // 2][qoff:qoff + 64, g * 512:(g + 1) * 512],
                            oT[:HD, :], bcs[:])

            # ---- projection: out^T[n, q] += w[c, n]^T-chunks @ x^T[c, q] ----
            for n in range(D // 128):
                for g in range(NG):
                    pr = ps512.tile([128, 512], f32, tag="S")
                    for cc in range(2):
                        nc.tensor.matmul(
                            pr[:],
                            w[cc][:, n * 128:(n + 1) * 128],
                            xT[cc][:, g * 512:(g + 1) * 512],
                            start=(cc == 0), stop=(cc == 1))
                    st = ptp.tile([128, 512], f32, tag="st")
                    nc.scalar.copy(st[:], pr[:])
                    nc.sync.dma_start(
                        d_outT[n * 128:(n + 1) * 128, g * 512:(g + 1) * 512],
                        st[:])

    nc.finalize()
    return nc


def _get_nc():
    if "nc" not in _cache:
        _cache["nc"] = _build()
    return _cache["nc"]


def _in_maps(q, k, v, w_out):
    cosT, sinT, rotP, maskT = _tables()
    ones = np.ones((T, 1), np.float32)
    maps = []
    for c in range(NCORES):
        b, kv = divmod(c, NKV)
        maps.append({
            "qT": np.ascontiguousarray(q[b, :, kv * CD:(kv + 1) * CD].T),
            "kT": np.ascontiguousarray(k[b, :, kv * KVD:(kv + 1) * KVD].T),
            "vaug": np.ascontiguousarray(
                np.concatenate([v[b, :, kv * KVD:(kv + 1) * KVD], ones], 1)),
            "w": np.ascontiguousarray(w_out[kv * CD:(kv + 1) * CD, :]),
            "cosT": cosT, "sinT": sinT, "rotP": rotP, "maskT": maskT,
            "ones64": np.ones((1, 64), np.float32),
        })
    return maps


def _run(q, k, v, w_out, trace=False):
    from concourse.bass_utils import run_bass_kernel_spmd

    nc = _get_nc()
    res = run_bass_kernel_spmd(nc, _in_maps(q, k, v, w_out),
                               core_ids=list(range(NCORES)), trace=trace)
    out = np.zeros((B, T, D), np.float32)
    for c in range(NCORES):
        out[c // NKV] += res.results[c]["outT"].T
    return out, res


def kernel(q, k, v, w_out):
    out, _ = _run(np.asarray(q), np.asarray(k), np.asarray(v),
                  np.asarray(w_out))
    return out


# revision 13
# speedup vs baseline: 1.2368x; 1.1587x over previous
"""GQA attention block (RoPE + causal softmax + out-projection) on 8 TRN2 cores.

Problem: q (2, 2048, 1024) 16 heads, k/v (2, 2048, 256) 4 kv heads (GQA rep 4),
causal attention, out @ w_out (1024, 1024).

Sharding: core c = (batch b = c//4, kv group = c%4). Each core computes its 4
q-heads x full T attention against its kv head, then the partial projection
X_heads @ w_out[head_rows, :]; the host sums the 4 partials per batch.

Layout trick: everything is computed transposed (channels on partitions,
sequence on the free axis):
  - S^T = K^T-block (64,128) stationary  @  Q^T (64, 512q) moving   (k on psum partitions)
  - P^T = exp(S^T/8) with causal handled by skipping kb blocks + one masked
    diagonal 128x128 add; no row-max (logits are O(1)) and no P normalization
  - O^T accumulates [V | 1] (128k, 65) stationary @ P^T moving, so the softmax
    denominator falls out as psum row 64; normalization = reciprocal +
    ones-matmul partition-broadcast + one multiply per (head, 512q)
  - projection: w-chunk (128c, 128n) stationary @ X^T (128c, 512q) moving,
    psum DMA'd straight to DRAM as out^T; host transposes while gathering.
RoPE runs on-chip: rotate_half is a signed-permutation matmul, the sin/cos
combine is 3 vector ops per 512-chunk against host-built per-partition tables.
All matmuls use float32r (1 cycle/row at N>=256, ~tf32 precision).
"""

import sys

if "/opt/trn_rl_repo" not in sys.path:
    sys.path.insert(0, "/opt/trn_rl_repo")

import numpy as np

B, T, D, NH, NKV, HD = 2, 2048, 1024, 16, 4, 64
HC = NH // NKV          # q heads per core = 4
CD = HC * HD            # per-core channel dim = 256
KVD = HD                # per-core kv channel dim = 64
NCORES = 8
QB = 128                # q/k block
NG = T // 512           # q column groups of 512
NKB = T // QB           # k blocks = 16
MASK = -240.0           # pre-scale additive mask; exp(-240/8) = exp(-30) ~ 1e-13

_cache: dict = {}


def _tables():
    if "tables" in _cache:
        return _cache["tables"]
    p = np.arange(128)
    t = np.arange(T)
    ang = t[None, :] / (10000.0 ** ((p[:, None] % 32) / 32.0))
    cosT = np.cos(ang).astype(np.float32)
    sinT = np.sin(ang).astype(np.float32)

    rotP = np.zeros((128, 128), np.float32)
    for base in (0, 64):
        for i in range(32):
            rotP[base + 32 + i, base + i] = -1.0   # out[i] = -x[i+32]
            rotP[base + i, base + 32 + i] = 1.0    # out[i+32] = x[i]

    kk = np.arange(QB)
    maskT = np.where(kk[:, None] <= kk[None, :], 0.0, MASK).astype(np.float32)
    _cache["tables"] = (cosT, sinT, rotP, maskT)
    return _cache["tables"]


def _build():
    import concourse.tile as tile
    from concourse import bacc, mybir

    f32 = mybir.dt.float32
    f32r = mybir.dt.float32r
    bf16 = mybir.dt.bfloat16
    Exp = mybir.ActivationFunctionType.Exp

    nc = bacc.Bacc("TRN2", target_bir_lowering=False, debug=False,
                   num_devices=NCORES)

    d_qT = nc.dram_tensor("qT", [CD, T], bf16, kind="ExternalInput")
    d_kT = nc.dram_tensor("kT", [KVD, T], bf16, kind="ExternalInput")
    d_vaug = nc.dram_tensor("vaug", [T, HD + 1], bf16, kind="ExternalInput")
    d_w = nc.dram_tensor("w", [CD, D], bf16, kind="ExternalInput")
    d_cosT = nc.dram_tensor("cosT", [128, T], bf16, kind="ExternalInput")
    d_sinT = nc.dram_tensor("sinT", [128, T], bf16, kind="ExternalInput")
    d_rotP = nc.dram_tensor("rotP", [128, 128], bf16, kind="ExternalInput")
    d_maskT = nc.dram_tensor("maskT", [QB, QB], f32, kind="ExternalInput")
    d_ones = nc.dram_tensor("ones64", [1, 64], f32r, kind="ExternalInput")
    d_outT = nc.dram_tensor("outT", [D, T], f32, kind="ExternalOutput")

    with tile.TileContext(nc) as tc:
        with (
            tc.tile_pool(name="consts", bufs=1) as consts,
            tc.tile_pool(name="data", bufs=1) as data,
            tc.tile_pool(name="pt", bufs=4) as ptp,
            tc.tile_pool(name="small", bufs=3) as small,
            tc.tile_pool(name="psS", bufs=3, space="PSUM") as psS,
            tc.tile_pool(name="psO", bufs=2, space="PSUM") as psO,
            tc.tile_pool(name="psB", bufs=1, space="PSUM") as psB,
        ):
            ps512 = psS  # rope/projection psum shares the S-bank slots
            cosT = consts.tile([128, T], bf16)
            sinT = consts.tile([128, T], bf16)
            rotP = consts.tile([128, 128], bf16)
            maskT = consts.tile([QB, QB], f32)
            ones64 = consts.tile([1, 64], f32r)
            nc.sync.dma_start(cosT[:], d_cosT[:])
            nc.sync.dma_start(sinT[:], d_sinT[:])
            nc.sync.dma_start(rotP[:], d_rotP[:])
            nc.sync.dma_start(maskT[:], d_maskT[:])
            nc.sync.dma_start(ones64[:], d_ones[:])

            qT = [data.tile([128, T], bf16, name=f"qT{i}", tag=f"qT{i}")
                  for i in range(2)]
            for i in range(2):
                nc.sync.dma_start(qT[i][:], d_qT[i * 128:(i + 1) * 128, :])
            kT = data.tile([KVD, T], bf16, tag="kT")
            nc.sync.dma_start(kT[:], d_kT[:])
            vaug = data.tile([128, NKB, HD + 1], bf16, tag="vaug")
            nc.sync.dma_start(
                vaug[:], d_vaug[:].rearrange("(n p) m -> p n m", p=128))
            w = [data.tile([128, D], bf16, name=f"w{i}", tag=f"w{i}")
                 for i in range(2)]
            for i in range(2):
                nc.sync.dma_start(w[i][:], d_w[i * 128:(i + 1) * 128, :])
            xT = [data.tile([128, T], bf16, name=f"xT{i}", tag=f"xT{i}")
                  for i in range(2)]

            # ---- RoPE (in-place on qT tiles and kT) ----
            def rope(dst, rows, tab_rows):
                for f in range(T // 512):
                    sl = slice(f * 512, (f + 1) * 512)
                    rot = ps512.tile([rows, 512], f32, tag="S")
                    nc.tensor.matmul(
                        rot[:], rotP[:rows, :rows],
                        dst[:, sl], start=True, stop=True)
                    nc.vector.tensor_mul(rot[:], rot[:], sinT[:tab_rows, sl])
                    nc.vector.tensor_mul(dst[:, sl], dst[:, sl],
                                         cosT[:tab_rows, sl])
                    nc.vector.tensor_add(dst[:, sl], dst[:, sl], rot[:])

            rope(qT[0], 128, 128)
            rope(qT[1], 128, 128)
            rope(kT, KVD, KVD)

            # second copy of roped K^T at partition base 64: matmul operands
            # must share a base partition, and odd heads' Q^T rows sit at 64
            kThi = data.tile([128, T], bf16, tag="kThi")
            nc.vector.tensor_copy(kThi[64:128, :], kT[:])

            # ---- attention: head pairs x 512-wide q groups, k-block inner
            # (two independent heads per k-block keep PE/ACT/DVE pipelined)
            for hp in range(HC // 2):
                heads = (2 * hp, 2 * hp + 1)
                for g in range(NG):
                    oTs = {h: psO.tile([HD + 1, 512], f32, name=f"oT{h}",
                                        tag=f"oT{h % 2}")
                           for h in heads}
                    nkb = 4 * g + 4
                    for kb in range(nkb):
                        cs = QB * max(0, kb - 4 * g)
                        for h in heads:
                            qoff = (h % 2) * 64
                            kTh = kT if qoff == 0 else kThi[64:128, :]
                            oT = oTs[h]
                            S = psS.tile([128, 512], f32)
                            nc.tensor.matmul(
                                S[:, cs:],
                                kTh[:, kb * QB:(kb + 1) * QB],
                                qT[h // 2][qoff:qoff + 64,
                                           g * 512 + cs:(g + 1) * 512],
                                start=True, stop=True)
                            if kb >= 4 * g:
                                nc.vector.tensor_add(S[:, cs:cs + QB],
                                                     S[:, cs:cs + QB],
                                                     maskT[:])
                            PT = ptp.tile([128, 512], bf16)
                            nc.scalar.activation(PT[:, cs:], S[:, cs:], Exp,
                                                 scale=0.125)
                            nc.tensor.matmul(
                                oT[:, cs:], vaug[:, kb, :],
                                PT[:, cs:],
                                start=(kb == 0), stop=(kb == nkb - 1),
                                skip_group_check=True)
                    for h in heads:
                        qoff = (h % 2) * 64
                        oT = oTs[h]
                        # normalize by psum row 64 (softmax denominator)
                        den = small.tile([1, 512], f32r, tag="den")
                        with nc.allow_low_precision("f32r softmax denom"):
                            nc.vector.reciprocal(den[:], oT[HD:HD + 1, :])
                        bc = psB.tile([64, 512], f32)
                        nc.tensor.matmul(bc[:], ones64[:],
                                         den[:], start=True, stop=True)
                        bcs = small.tile([64, 512], f32, tag="bcs")
                        nc.vector.tensor_copy(bcs[:], bc[:])
                        nc.vector.tensor_mul(
                            xT[h // 2][qoff:qoff + 64, g * 512:(g + 1) * 512],
                            oT[:HD, :], bcs[:])

            # ---- projection: out^T[n, q] += w[c, n]^T-chunks @ x^T[c, q] ----
            for n in range(D // 128):
                for g in range(NG):
                    pr = ps512.tile([128, 512], f32, tag="S")
                    for cc in range(2):
                        nc.tensor.matmul(
                            pr[:],
                            w[cc][:, n * 128:(n + 1) * 128],
                            xT[cc][:, g * 512:(g + 1) * 512],
                            start=(cc == 0), stop=(cc == 1))
                    st = ptp.tile([128, 512], f32, tag="st")
                    nc.scalar.copy(st[:], pr[:])
                    nc.sync.dma_start(
                        d_outT[n * 128:(n + 1) * 128, g * 512:(g + 1) * 512],
                        st[:])

    nc.finalize()
    return nc


def _get_nc():
    if "nc" not in _cache:
        _cache["nc"] = _build()
    return _cache["nc"]


def _in_maps(q, k, v, w_out):
    import ml_dtypes
    bf = ml_dtypes.bfloat16
    cosT, sinT, rotP, maskT = _tables()
    ones = np.ones((T, 1), np.float32)
    maps = []
    for c in range(NCORES):
        b, kv = divmod(c, NKV)
        maps.append({
            "qT": np.ascontiguousarray(q[b, :, kv * CD:(kv + 1) * CD].T).astype(bf),
            "kT": np.ascontiguousarray(k[b, :, kv * KVD:(kv + 1) * KVD].T).astype(bf),
            "vaug": np.ascontiguousarray(
                np.concatenate([v[b, :, kv * KVD:(kv + 1) * KVD], ones], 1)).astype(bf),
            "w": np.ascontiguousarray(w_out[kv * CD:(kv + 1) * CD, :]).astype(bf),
            "cosT": cosT.astype(bf), "sinT": sinT.astype(bf),
            "rotP": rotP.astype(bf), "maskT": maskT,
            "ones64": np.ones((1, 64), np.float32),
        })
    return maps


def _run(q, k, v, w_out, trace=False):
    from concourse.bass_utils import run_bass_kernel_spmd

    nc = _get_nc()
    res = run_bass_kernel_spmd(nc, _in_maps(q, k, v, w_out),
                               core_ids=list(range(NCORES)), trace=trace)
    out = np.zeros((B, T, D), np.float32)
    for c in range(NCORES):
        out[c // NKV] += res.results[c]["outT"].T
    return out, res


def kernel(q, k, v, w_out):
    out, _ = _run(np.asarray(q), np.asarray(k), np.asarray(v),
                  np.asarray(w_out))
    return out
